# revision 13
# baseline (speedup 1.0000x reference)
"""Causal self-attention on 8 TRN2 NeuronCores.

Reference computation (B=4, T=2048, C=1024, H=16 heads, hd=64):
    qkv = x @ W_attn + b_attn ; split q,k,v ; per-head causal softmax attention
    y = att @ v ; out = y @ W_proj + b_proj

Sharding: core = 2*b + g  (b = batch 0..3, g = head-half 0..1, heads 8g..8g+7).
Each core computes its batch's Q/K/V for its 8 heads, flash-style causal
attention entirely in SBUF, and a partial out^T = Wp_slice^T @ y^T.  The two
cores of a batch produce partials that the host sums (pure data movement +
one add); host also re-transposes to [B,T,C].

Layouts are feature-major (x^T, Q^T, K^T, y^T, out^T) so no transposes are
needed on device.  S^T[k,q] = K^T.T @ Q^T puts softmax on the partition axis;
the denominator comes free from an appended ones-column on V (M=65 matmul).
All matmuls run in float32r (full PE speed, ~1.5e-4 median rel err).

Schedule: emission interleaves three streams so the PE never idles (HAM
stays warm) — qkv projection of t-tile j+1 and earlier out-projections are
woven between the attention steps of q-wave j; within a wave the PV matmul
of step k-LAG is emitted next to the S matmul of step k so the
S -> exp -> mask -> PV chain never stalls the PE.  The two head-parities of
a step share one two-bank PSUM tile so exp+mask run as single instructions
(halves ACT instruction+semaphore count).  Softmax normalization transposes
the sums row to [128,4] via a DRAM bounce so the microcoded reciprocal runs
lane-parallel, then broadcasts the reciprocals back via a stride-0 read.
"""

import numpy as np

B, T, C, H = 4, 2048, 1024, 16
HD = C // H          # 64
HPC = 8              # heads per core
NCORES = 8
TQ = 512             # q tile (free dim / psum bank)
NQT = T // TQ        # 4 q tiles (waves) per batch
NCC = C // 128       # 8 contraction chunks of 128
LAG = 2              # PV lags S by this many attention steps

_cache = {}


def _build():
    if "nc" in _cache:
        return _cache["nc"]

    import concourse.bass as bass
    import concourse.tile as tile
    from concourse import bacc, mybir

    F32 = mybir.dt.float32
    F32R = mybir.dt.float32r
    AF = mybir.ActivationFunctionType

    nc = bacc.Bacc("TRN2", target_bir_lowering=False, debug=False,
                   num_devices=NCORES)

    xt_d = nc.dram_tensor("xt", [C, T], F32, kind="ExternalInput").ap()
    wqk_d = nc.dram_tensor("wqk", [C, 1024], F32, kind="ExternalInput").ap()
    wv_d = nc.dram_tensor("wv", [C, 512], F32, kind="ExternalInput").ap()
    wp_d = nc.dram_tensor("wp", [512, C], F32, kind="ExternalInput").ap()
    bqk_d = nc.dram_tensor("bqk", [1024], F32, kind="ExternalInput").ap()
    bout_d = nc.dram_tensor("bout", [C], F32, kind="ExternalInput").ap()
    tri_d = nc.dram_tensor("tri", [128, 256], F32, kind="ExternalInput").ap()
    outp_d = nc.dram_tensor("outp", [C, T], F32, kind="ExternalOutput").ap()

    with tile.TileContext(nc) as tc:
        import contextlib
        stack = contextlib.ExitStack()
        with stack:
            singles = stack.enter_context(tc.tile_pool(name="singles", bufs=1))
            ps = stack.enter_context(tc.tile_pool(name="ps", space="PSUM",
                                                  bufs=1))
            qpool = stack.enter_context(tc.tile_pool(name="qpool", bufs=2))
            ypool = stack.enter_context(tc.tile_pool(name="ypool", bufs=2))
            xtp = stack.enter_context(tc.tile_pool(name="xtp", bufs=1))
            ppool = stack.enter_context(tc.tile_pool(name="ppool", bufs=3))
            bcp = stack.enter_context(tc.tile_pool(name="bcp", bufs=2))
            ostp = stack.enter_context(tc.tile_pool(name="ostp", bufs=2))
            ystg = stack.enter_context(tc.tile_pool(name="ystg", bufs=2))
            drp = stack.enter_context(tc.tile_pool(name="drp", bufs=16,
                                                   space="DRAM"))

            tri_sb = singles.tile([128, 2, 128], F32R)
            nc.gpsimd.dma_start(
                out=tri_sb,
                in_=tri_d.rearrange("p (a q) -> p a q", a=2).bitcast(F32R))
            bqk_sb = singles.tile([128, 8], F32)
            nc.gpsimd.dma_start(out=bqk_sb,
                                in_=bqk_d.rearrange("(c p) -> p c", p=128))
            bout_sb = singles.tile([128, 8], F32)
            nc.gpsimd.dma_start(out=bout_sb,
                                in_=bout_d.rearrange("(c p) -> p c", p=128))

            # K^T resident: [feat128, pair, t];  V: [t128, kchunk, head, 65]
            k_sb = singles.tile([128, 4, T], F32R)
            v_sb = singles.tile([128, T // 128, HPC, 65], F32R)
            ones_sb = singles.tile([128, (T // 128) * HPC], F32)
            nc.vector.memset(ones_sb, 1.0)
            nc.vector.tensor_copy(
                v_sb[:, :, :, 64],
                ones_sb.rearrange("p (a b) -> p a b", a=T // 128))

            wqk_sb = singles.tile([128, NCC, 1024], F32R)
            wv_sb = singles.tile([128, NCC, 512], F32R)
            wp_sb = singles.tile([128, 4, 1024], F32R)
            wqk_r = wqk_d.rearrange("(c p) m -> p c m", p=128).bitcast(F32R)
            wv_r = wv_d.rearrange("(c p) m -> p c m", p=128).bitcast(F32R)
            wp_r = wp_d.rearrange("(c p) m -> p c m", p=128).bitcast(F32R)
            for c in range(NCC):
                nc.gpsimd.dma_start(out=wv_sb[:, c, :], in_=wv_r[:, c, :])
            for c in range(4):
                nc.gpsimd.dma_start(out=wp_sb[:, c, :], in_=wp_r[:, c, :])

            q_tiles = {}   # wave j -> [128, 4, TQ] tile
            y_tiles = {}   # keys (j, cp) psum pair, (j, "sb") sbuf tile

            # ---------- emission closures ----------
            def qkv_groups(tt):
                """13 emission closures for t-tile tt of the projections."""
                xt = [None]

                def load_x():
                    xt[0] = xtp.tile([128, NCC, TQ], F32R, tag="xt",
                                     name=f"xt_{tt}")
                    xr = xt_d.rearrange("(c p) t -> p c t", p=128) \
                        [:, :, tt * TQ:(tt + 1) * TQ].bitcast(F32R)
                    for c in range(NCC):
                        nc.sync.dma_start(out=xt[0][:, c, :], in_=xr[:, c, :])
                    if tt == 0:
                        for c in range(NCC):
                            nc.sync.dma_start(out=wqk_sb[:, c, :],
                                              in_=wqk_r[:, c, :])
                    q_tiles[tt] = qpool.tile([128, 4, TQ], F32R, tag="q",
                                             name=f"q_{tt}")

                def qk_chunk(m):
                    def emit():
                        acc = ps.tile([128, TQ], F32, tag="acc", bufs=2,
                                      name=f"acc_qk_{tt}_{m}")
                        for c in range(NCC):
                            nc.tensor.matmul(
                                acc, wqk_sb[:, c, m * 128:(m + 1) * 128],
                                xt[0][:, c, :],
                                start=(c == 0), stop=(c == NCC - 1))
                        if m < 4:
                            dst = q_tiles[tt][:, m, :]
                        else:
                            dst = k_sb[:, m - 4, tt * TQ:(tt + 1) * TQ]
                        nc.vector.tensor_scalar_add(dst, acc,
                                                    bqk_sb[:, m:m + 1])
                    return emit

                def v_chunk(v4):
                    def emit():
                        ti = tt * 4 + v4
                        acc = ps.tile([128, TQ], F32, tag="acc", bufs=2,
                                      name=f"acc_v_{tt}_{v4}")
                        for c in range(NCC):
                            nc.tensor.matmul(
                                acc, xt[0][:, c, v4 * 128:(v4 + 1) * 128],
                                wv_sb[:, c, :],
                                start=(c == 0), stop=(c == NCC - 1))
                        nc.vector.tensor_copy(
                            v_sb[:, ti, :, 0:64],
                            acc.rearrange("p (h d) -> p h d", h=HPC))
                    return emit

                out = [load_x]
                for v4 in range(4):
                    out.append(v_chunk(v4))
                for m in range(8):
                    out.append(qk_chunk(m))
                return out

            def attention_wave(j):
                """Emission closures for q-wave j: pipelined S/exp/PV with
                LAG, plus normalization per head-pair."""
                nkc = 4 * j + 4
                steps = [(cp, i) for cp in range(4) for i in range(nkc)]
                pend = {}

                def emit_S(k):
                    cp, i = steps[k]
                    r = max(0, (i - 4 * j) * 128)
                    s_ps = ps.tile([128, 2, TQ], F32, tag="s", bufs=2,
                                   name=f"s_{j}_{k}")
                    for par in range(2):
                        row0 = 64 * par
                        nc.tensor.matmul(
                            s_ps[:, par, r:TQ],
                            k_sb[row0:row0 + 64, cp, i * 128:(i + 1) * 128],
                            q_tiles[j][row0:row0 + 64, cp, r:TQ],
                            start=True, stop=True, tile_position=(row0, 0))
                    p_sb = ppool.tile([128, 2, TQ], F32R, tag="p",
                                      name=f"p_{j}_{k}")
                    nc.scalar.activation(p_sb[:, :, r:TQ], s_ps[:, :, r:TQ],
                                         AF.Exp)
                    if i >= 4 * j:
                        nc.vector.tensor_mul(p_sb[:, :, r:r + 128],
                                             p_sb[:, :, r:r + 128], tri_sb)
                    pend[k] = (r, p_sb)

                def emit_PV(k):
                    cp, i = steps[k]
                    r, p_sb = pend.pop(k)
                    if i == 0:
                        y_tiles[(j, cp)] = [
                            ps.tile([65, TQ], F32, tag="y", bufs=2,
                                    name=f"yps_{j}_{cp}_{par}")
                            for par in range(2)]
                    for par in range(2):
                        nc.tensor.matmul(
                            y_tiles[(j, cp)][par][:, r:TQ],
                            v_sb[:, i, 2 * cp + par, :], p_sb[:, par, r:TQ],
                            start=(i == 0), stop=(i == nkc - 1))
                    if i == nkc - 1:
                        emit_norm(cp)

                def emit_norm(cp):
                    dma = nc.gpsimd.dma_start if cp % 2 == 0 else \
                        nc.sync.dma_start
                    ysts = []
                    for par in range(2):
                        y_ps = y_tiles[(j, cp)][par]
                        # free the psum bank fast: one copy takes y + sums row
                        yst = ystg.tile([65, TQ], F32, tag="yst",
                                        name=f"yst_{j}_{cp}_{par}")
                        nc.vector.tensor_copy(yst, y_ps)
                        ysts.append(yst)
                    d1 = drp.tile([2, TQ], F32, tag="d1",
                                  name=f"d1_{j}_{cp}")
                    for par in range(2):
                        dma(out=d1[par:par + 1, :], in_=ysts[par][64:65, :])
                    s4 = bcp.tile([128, 2, 4], F32, tag="s4",
                                  name=f"s4_{j}_{cp}")
                    dma(out=s4,
                        in_=bass.AP(tensor=d1.tensor, offset=d1.offset,
                                    ap=[[4, 128], [TQ, 2], [1, 4]]))
                    r4 = bcp.tile([128, 2, 4], F32, tag="r4",
                                  name=f"r4_{j}_{cp}")
                    nc.vector.reciprocal(r4, s4)
                    d2 = drp.tile([128, 2, 4], F32, tag="d2",
                                  name=f"d2_{j}_{cp}")
                    dma(out=d2, in_=r4)
                    for par in range(2):
                        row0 = 64 * par
                        bc = bcp.tile([64, TQ], F32, tag="bc",
                                      name=f"bc_{j}_{cp}_{par}")
                        dma(out=bc,
                            in_=bass.AP(tensor=d2.tensor,
                                        offset=d2.offset + par * 4,
                                        ap=[[0, 64], [8, TQ // 4], [1, 4]]))
                        nc.vector.tensor_mul(
                            y_tiles[(j, "sb")][row0:row0 + 64, cp, :],
                            ysts[par][0:64, :], bc)

                def step(k):
                    def emit():
                        if k == 0:
                            y_tiles[(j, "sb")] = ypool.tile(
                                [128, 4, TQ], F32R, tag="ysb", name=f"y_{j}")
                        if k < len(steps):
                            emit_S(k)
                        if k >= LAG:
                            emit_PV(k - LAG)
                    return emit

                return [step(k) for k in range(len(steps) + LAG)]

            def proj_groups(j, tag="acc"):
                def chunk(mo):
                    def emit():
                        shape = [128, TQ] if tag == "acc" else [128, 2, TQ]
                        accf = ps.tile(shape, F32, tag=tag, bufs=2,
                                       name=f"acc_pr_{j}_{mo}")
                        acc = accf if tag == "acc" else accf[:, 0, :]
                        for c in range(4):
                            nc.tensor.matmul(
                                acc, wp_sb[:, c, mo * 128:(mo + 1) * 128],
                                y_tiles[(j, "sb")][:, c, :],
                                start=(c == 0), stop=(c == 3))
                        ot = ostp.tile([128, TQ], F32, tag="ot",
                                       name=f"ot_{j}_{mo}")
                        nc.vector.tensor_scalar_add(ot, acc,
                                                    bout_sb[:, mo:mo + 1])
                        nc.sync.dma_start(
                            out=outp_d[mo * 128:(mo + 1) * 128,
                                       j * TQ:(j + 1) * TQ],
                            in_=ot)
                    return emit
                return [chunk(mo) for mo in range(8)]

            # ---------- interleaved emission ----------
            for fn in qkv_groups(0):
                fn()
            for j in range(NQT):
                attn = attention_wave(j)
                others = []
                if j + 1 < NQT:
                    others += qkv_groups(j + 1)
                if j >= 1:
                    others += proj_groups(j - 1)
                done_o = 0
                for s, fn in enumerate(attn):
                    fn()
                    want = (s + 1) * len(others) // len(attn)
                    while done_o < want:
                        others[done_o]()
                        done_o += 1
                while done_o < len(others):
                    others[done_o]()
                    done_o += 1
            for fn in proj_groups(NQT - 1, tag="s"):
                fn()

    nc.compile()
    _cache["nc"] = nc
    return nc


def _prep_inputs(x, W_attn, b_attn, W_proj, b_proj):
    """Host-side sharding: returns in_maps for the 8 cores."""
    x = np.ascontiguousarray(np.asarray(x, dtype=np.float32))
    W_attn = np.asarray(W_attn, dtype=np.float32)
    b_attn = np.asarray(b_attn, dtype=np.float32)
    W_proj = np.asarray(W_proj, dtype=np.float32)
    b_proj = np.asarray(b_proj, dtype=np.float32)

    tri1 = np.triu(np.ones((128, 128), dtype=np.float32))  # 1 if k<=q
    tri = np.concatenate([tri1, tri1], axis=1)

    xts = [np.ascontiguousarray(x[b].T) for b in range(B)]
    per_g = []
    for g in range(2):
        sl = slice(512 * g, 512 * (g + 1))
        wq = W_attn[:, 0:C][:, sl] * (1.0 / np.sqrt(HD))
        wk = W_attn[:, C:2 * C][:, sl]
        wv = W_attn[:, 2 * C:3 * C][:, sl]
        bq = b_attn[0:C][sl] * (1.0 / np.sqrt(HD))
        bk = b_attn[C:2 * C][sl]
        bv = b_attn[2 * C:3 * C][sl]
        wp = W_proj[sl, :]
        bout = b_proj * 0.5 + bv @ wp
        per_g.append({
            "wqk": np.ascontiguousarray(np.concatenate([wq, wk], axis=1)),
            "wv": np.ascontiguousarray(wv),
            "wp": np.ascontiguousarray(wp),
            "bqk": np.ascontiguousarray(np.concatenate([bq, bk])),
            "bout": np.ascontiguousarray(bout.astype(np.float32)),
        })

    in_maps = []
    for b in range(B):
        for g in range(2):
            m = dict(per_g[g])
            m["xt"] = xts[b]
            m["tri"] = tri
            in_maps.append(m)
    return in_maps


def run_sharded(x, W_attn, b_attn, W_proj, b_proj, trace=False):
    """Run on 8 cores; returns (output [B,T,C], BassKernelResults)."""
    from concourse.bass_utils import run_bass_kernel_spmd

    nc = _build()
    in_maps = _prep_inputs(x, W_attn, b_attn, W_proj, b_proj)
    res = run_bass_kernel_spmd(nc, in_maps, list(range(NCORES)), trace=trace)
    outs = [res.results[i]["outp"] for i in range(NCORES)]
    out = np.empty((B, T, C), dtype=np.float32)
    for b in range(B):
        out[b] = (outs[2 * b] + outs[2 * b + 1]).T
    return out, res


def kernel(x, W_attn, b_attn, W_proj, b_proj):
    out, _ = run_sharded(x, W_attn, b_attn, W_proj, b_proj, trace=False)
    return out
